# revision 1
# baseline (speedup 1.0000x reference)
"""Sparse-attention kernel (nn_Attention_65180423685537) on 8 Trainium2 NeuronCores.

Strategy: pure data-parallel over batch B=8 -> one batch element per core.
All attention / window partitioning is batch-independent; small projection
weights are replicated to every core. Inputs are taken FULL, sharded
internally over the 8 devices, and the full-shape outputs are gathered.

Self-contained: shapes/sharding hardcoded, no sibling imports.
"""

import numpy as np

DIM = 128
HEADS = 8
HD = DIM // HEADS          # 16
SR = 2
WS = 7
SCALE = HD ** -0.5
LN_EPS = 1e-5
B, H, W = 8, 56, 56
N, C = H * W, DIM

_WKEYS = ["W_lepe", "b_lepe", "W_dw", "b_dw", "W_sr", "b_sr", "g_ln", "b_ln",
          "W_q1", "W_kv1", "W_q2", "W_kv2", "W_proj", "b_proj"]

_compiled = None


def _build():
    global _compiled
    if _compiled is not None:
        return _compiled

    import jax
    import jax.numpy as jnp
    from jax.sharding import Mesh, PartitionSpec as P
    try:
        from jax.experimental.shard_map import shard_map
    except ImportError:
        from jax.shard_map import shard_map

    nh2 = HEADS // 2

    def _win_part(x, ws):
        Bh, n, d = x.shape
        x = x.reshape(Bh, H // ws, ws, W // ws, ws, d).transpose(0, 1, 3, 2, 4, 5)
        return x.reshape(-1, ws * ws, d)

    def _win_rev(w, ws, Bh):
        d = w.shape[-1]
        x = w.reshape(Bh, H // ws, W // ws, ws, ws, d).transpose(0, 1, 3, 2, 4, 5)
        return x.reshape(Bh, H * W, d)

    def body(x, W_lepe, b_lepe, W_dw, b_dw, W_sr, b_sr, g_ln, b_ln,
             W_q1, W_kv1, W_q2, W_kv2, W_proj, b_proj):
        # x: [b_local, N, C] with b_local = 1 per core
        Bl = x.shape[0]

        # --- LePE: linear -> depthwise 3x3 conv ---
        l = (x @ W_lepe + b_lepe).transpose(0, 2, 1).reshape(Bl, C, H, W)
        l = jax.lax.conv_general_dilated(
            l, W_dw, (1, 1), ((1, 1), (1, 1)), feature_group_count=C
        ) + b_dw[None, :, None, None]
        lepe = l.reshape(Bl, C, N).transpose(0, 2, 1)

        # --- branch 1: global attention over sr-downsampled kv ---
        q1 = (x @ W_q1).reshape(Bl, N, nh2, HD).transpose(0, 2, 1, 3)
        xs = x.transpose(0, 2, 1).reshape(Bl, C, H, W)
        xs = jax.lax.conv_general_dilated(xs, W_sr, (SR, SR), 'VALID') \
            + b_sr[None, :, None, None]
        Hs, Ws = H // SR, W // SR
        xs = xs.reshape(Bl, C, Hs * Ws).transpose(0, 2, 1)
        mu = xs.mean(-1, keepdims=True)
        var = ((xs - mu) ** 2).mean(-1, keepdims=True)
        xs = (xs - mu) * jax.lax.rsqrt(var + LN_EPS) * g_ln + b_ln
        xs = jax.nn.gelu(xs, approximate=False)
        kv1 = (xs @ W_kv1).reshape(Bl, Hs * Ws, 2, nh2, HD).transpose(2, 0, 3, 1, 4)
        k1, v1 = kv1[0], kv1[1]
        attn1 = jax.nn.softmax(
            jnp.einsum('bhqd,bhkd->bhqk', q1, k1) * SCALE, axis=-1)
        x1 = jnp.einsum('bhqk,bhkd->bhqd', attn1, v1)
        x1 = x1.transpose(0, 2, 1, 3).reshape(Bl, N, C // 2)
        gm = attn1.mean(1).mean(1).reshape(Bl, Hs, Ws)
        gm = jnp.repeat(jnp.repeat(gm, SR, axis=1), SR, axis=2)

        # --- branch 2: 7x7 windowed attention ---
        q2 = (x @ W_q2).reshape(Bl, N, nh2, HD).transpose(0, 2, 1, 3)
        q2 = q2.reshape(Bl * nh2, N, HD)
        kv2 = (x @ W_kv2).reshape(Bl, N, 2, nh2, HD).transpose(2, 0, 3, 1, 4)
        k2 = kv2[0].reshape(Bl * nh2, N, HD)
        v2 = kv2[1].reshape(Bl * nh2, N, HD)
        q2w = _win_part(q2, WS)
        k2w = _win_part(k2, WS)
        v2w = _win_part(v2, WS)
        attn2 = jax.nn.softmax(
            jnp.einsum('wqd,wkd->wqk', q2w, k2w) * SCALE, axis=-1)
        x2w = jnp.einsum('wqk,wkd->wqd', attn2, v2w)
        x2 = _win_rev(x2w, WS, Bl * nh2)
        x2 = x2.reshape(Bl, nh2, N, HD).transpose(0, 2, 1, 3).reshape(Bl, N, C // 2)
        nwH, nwW = H // WS, W // WS
        a2 = attn2.reshape(Bl, nh2, nwH * nwW, WS * WS, WS * WS)
        lm = a2.mean(axis=(1, 3, 4)).reshape(Bl, nwH, nwW)
        lm = jnp.repeat(jnp.repeat(lm, WS, axis=1), WS, axis=2)

        out = jnp.concatenate([x1, x2], axis=-1)
        out = (out + lepe) @ W_proj + b_proj

        mask = lm + gm
        mask_1 = mask.reshape(Bl, H * W)
        mask_2 = mask.transpose(0, 2, 1).reshape(Bl, H * W)
        return out, mask_1, mask_2

    devices = jax.devices()[:8]
    mesh = Mesh(np.asarray(devices), ("b",))
    in_specs = (P("b"),) + (P(),) * len(_WKEYS)
    out_specs = (P("b"), P("b"), P("b"))
    fn = jax.jit(shard_map(body, mesh=mesh, in_specs=in_specs,
                           out_specs=out_specs, check_rep=False))
    _compiled = fn
    return fn


def kernel(**inputs):
    fn = _build()
    x = np.ascontiguousarray(np.asarray(inputs["x"], dtype=np.float32))
    ws = [np.ascontiguousarray(np.asarray(inputs[k], dtype=np.float32))
          for k in _WKEYS]
    out, m1, m2 = fn(x, *ws)
    return (np.asarray(out, dtype=np.float32),
            np.asarray(m1, dtype=np.float32),
            np.asarray(m2, dtype=np.float32))


# revision 3
# speedup vs baseline: 7.0209x; 7.0209x over previous
"""Sparse-attention kernel (nn_Attention_65180423685537) on 8 Trainium2 NeuronCores.

Strategy: pure data-parallel over batch B=8 -> one batch element per core.
All attention / window partitioning is batch-independent; small projection
weights are replicated to every core. Inputs are taken FULL, sharded
internally over the 8 devices, and the full-shape outputs are gathered.

Self-contained: shapes/sharding hardcoded, no sibling imports.
"""

import numpy as np

DIM = 128
HEADS = 8
HD = DIM // HEADS          # 16
SR = 2
WS = 7
SCALE = HD ** -0.5
LN_EPS = 1e-5
B, H, W = 8, 56, 56
N, C = H * W, DIM

_WKEYS = ["W_lepe", "b_lepe", "W_dw", "b_dw", "W_sr", "b_sr", "g_ln", "b_ln",
          "W_q1", "W_kv1", "W_q2", "W_kv2", "W_proj", "b_proj"]

_compiled = None


def _build():
    global _compiled
    if _compiled is not None:
        return _compiled

    import jax
    import jax.numpy as jnp
    from jax.sharding import Mesh, PartitionSpec as P
    try:
        from jax.experimental.shard_map import shard_map
    except ImportError:
        from jax.shard_map import shard_map

    nh2 = HEADS // 2

    def _win_part(x, ws):
        Bh, n, d = x.shape
        x = x.reshape(Bh, H // ws, ws, W // ws, ws, d).transpose(0, 1, 3, 2, 4, 5)
        return x.reshape(-1, ws * ws, d)

    def _win_rev(w, ws, Bh):
        d = w.shape[-1]
        x = w.reshape(Bh, H // ws, W // ws, ws, ws, d).transpose(0, 1, 3, 2, 4, 5)
        return x.reshape(Bh, H * W, d)

    bf = jnp.bfloat16
    f32 = jnp.float32

    def mm(a, b):
        # bf16 inputs, f32 accumulation: Trainium PE streams bf16 at 4x the
        # fp32 rate; rel-err budget (2e-2) absorbs the input rounding.
        return jnp.matmul(a.astype(bf), b.astype(bf),
                          preferred_element_type=f32)

    def ein(spec, a, b):
        return jnp.einsum(spec, a.astype(bf), b.astype(bf),
                          preferred_element_type=f32)

    def body(x, W_lepe, b_lepe, W_dw, b_dw, W_sr, b_sr, g_ln, b_ln,
             W_q1, W_kv1, W_q2, W_kv2, W_proj, b_proj):
        # x: [b_local, N, C] with b_local = 1 per core
        Bl = x.shape[0]

        # --- LePE: linear -> depthwise 3x3 conv ---
        l = (mm(x, W_lepe) + b_lepe).transpose(0, 2, 1).reshape(Bl, C, H, W)
        l = jax.lax.conv_general_dilated(
            l, W_dw, (1, 1), ((1, 1), (1, 1)), feature_group_count=C
        ) + b_dw[None, :, None, None]
        lepe = l.reshape(Bl, C, N).transpose(0, 2, 1)

        # --- branch 1: global attention over sr-downsampled kv ---
        q1 = mm(x, W_q1).reshape(Bl, N, nh2, HD).transpose(0, 2, 1, 3)
        xs = x.transpose(0, 2, 1).reshape(Bl, C, H, W)
        xs = jax.lax.conv_general_dilated(xs, W_sr, (SR, SR), 'VALID') \
            + b_sr[None, :, None, None]
        Hs, Ws = H // SR, W // SR
        xs = xs.reshape(Bl, C, Hs * Ws).transpose(0, 2, 1)
        mu = xs.mean(-1, keepdims=True)
        var = ((xs - mu) ** 2).mean(-1, keepdims=True)
        xs = (xs - mu) * jax.lax.rsqrt(var + LN_EPS) * g_ln + b_ln
        xs = jax.nn.gelu(xs, approximate=False)
        kv1 = mm(xs, W_kv1).reshape(Bl, Hs * Ws, 2, nh2, HD).transpose(2, 0, 3, 1, 4)
        k1, v1 = kv1[0], kv1[1]
        attn1 = jax.nn.softmax(
            ein('bhqd,bhkd->bhqk', q1, k1) * SCALE, axis=-1)
        x1 = ein('bhqk,bhkd->bhqd', attn1, v1)
        x1 = x1.transpose(0, 2, 1, 3).reshape(Bl, N, C // 2)
        gm = attn1.mean(1).mean(1).reshape(Bl, Hs, Ws)
        gm = jnp.repeat(jnp.repeat(gm, SR, axis=1), SR, axis=2)

        # --- branch 2: 7x7 windowed attention ---
        q2 = mm(x, W_q2).reshape(Bl, N, nh2, HD).transpose(0, 2, 1, 3)
        q2 = q2.reshape(Bl * nh2, N, HD)
        kv2 = mm(x, W_kv2).reshape(Bl, N, 2, nh2, HD).transpose(2, 0, 3, 1, 4)
        k2 = kv2[0].reshape(Bl * nh2, N, HD)
        v2 = kv2[1].reshape(Bl * nh2, N, HD)
        q2w = _win_part(q2, WS)
        k2w = _win_part(k2, WS)
        v2w = _win_part(v2, WS)
        attn2 = jax.nn.softmax(
            ein('wqd,wkd->wqk', q2w, k2w) * SCALE, axis=-1)
        x2w = ein('wqk,wkd->wqd', attn2, v2w)
        x2 = _win_rev(x2w, WS, Bl * nh2)
        x2 = x2.reshape(Bl, nh2, N, HD).transpose(0, 2, 1, 3).reshape(Bl, N, C // 2)
        nwH, nwW = H // WS, W // WS
        a2 = attn2.reshape(Bl, nh2, nwH * nwW, WS * WS, WS * WS)
        lm = a2.mean(axis=(1, 3, 4)).reshape(Bl, nwH, nwW)
        lm = jnp.repeat(jnp.repeat(lm, WS, axis=1), WS, axis=2)

        out = jnp.concatenate([x1, x2], axis=-1)
        out = mm(out + lepe, W_proj) + b_proj

        mask = lm + gm
        mask_1 = mask.reshape(Bl, H * W)
        mask_2 = mask.transpose(0, 2, 1).reshape(Bl, H * W)
        return out, mask_1, mask_2

    devices = jax.devices()[:8]
    mesh = Mesh(np.asarray(devices), ("b",))
    in_specs = (P("b"),) + (P(),) * len(_WKEYS)
    out_specs = (P("b"), P("b"), P("b"))
    fn = jax.jit(shard_map(body, mesh=mesh, in_specs=in_specs,
                           out_specs=out_specs, check_rep=False))
    _compiled = fn
    return fn


def kernel(**inputs):
    fn = _build()
    x = np.ascontiguousarray(np.asarray(inputs["x"], dtype=np.float32))
    ws = [np.ascontiguousarray(np.asarray(inputs[k], dtype=np.float32))
          for k in _WKEYS]
    out, m1, m2 = fn(x, *ws)
    return (np.asarray(out, dtype=np.float32),
            np.asarray(m1, dtype=np.float32),
            np.asarray(m2, dtype=np.float32))
